# revision 1
# baseline (speedup 1.0000x reference)
"""Trainium2 Bass kernel for the Tolles-Lawson custom loss.

reference:
    c = model_output[:, :18]; d = model_output[:, 18:19]
    tmp = sum(A * (beta_TL + c), axis=1, keepdims=True) + d
    L = mean((tmp - y)^2) + mean((tmp - B_tl)^2)

Sharding: pure data parallel over rows on 8 cores (R = 501,760 rows per
core, core 7 zero-padded; zero rows contribute 0 to both sums). The two
MSE sums are reduced on-chip to per-core partials [128, 2*NT]; the
all-reduce across shards is done host-side (the output is one scalar).

Layout (per core): partition p holds rows [p*3920, (p+1)*3920) of the
shard as contiguous per-partition runs. Three bf16 streams:
  moc [128, 3920*18]  model_output[:, :18]
  a   [128, 3920*18]  A
  dyb [128, 3920*3]   interleaved [d, y, B_tl] records
Tolerance note: inputs are cast to bf16 (2e-2 rel tolerance; per-row
rounding error is random and washes out in the 4M-row mean; measured
rel err ~6e-4). All accumulation is f32.

Per tile (T=245 rows/partition, 16 tiles), DMAs in 2-tile chunks
(2.26 MB transfers via HWDGE):
  DVE : bc = moc + vb (bf16 2x), prod = a * bc (2x),
        tree-reduce 18->8(+2)->4->2->1 (2x except last fold),
        tmp += d, e1 = tmp - y, e2 = tmp - b
  ACT : de-interleave d/y/b from dyb, Square+row-accumulate e1/e2
        into accs[:, 2i], accs[:, 2i+1]
  (GPSIMD deliberately unused: measured ~4x slower than its cost model
  and it serializes with DVE on shared tiles.)

Measured (For_i slope, 8 cores): ~153 us/pass vs 475 us baseline.
"""

import numpy as np
import ml_dtypes

import concourse.bacc as bacc
import concourse.mybir as mybir
from concourse import tile
from concourse.bass_utils import run_bass_kernel_spmd

N_TOTAL = 4_000_000
NCOEF = 18
P = 128
T = 245          # rows per partition per compute tile
NT = 16          # compute tiles per core
RP = T * NT      # 3920 rows per partition
R = P * RP       # 501,760 rows per core
N_CORES = 8
CHUNK = 2        # compute tiles per DMA transfer

f32 = mybir.dt.float32
bf16 = mybir.dt.bfloat16
add = mybir.AluOpType.add
sub = mybir.AluOpType.subtract
mult = mybir.AluOpType.mult
SQ = mybir.ActivationFunctionType.Square

_cached = {}


def _build(hw_rep=0, io_bufs=3, work_bufs=3):
    """hw_rep > 0 wraps the pass in a For_i hardware loop (bench only)."""
    key = (hw_rep, io_bufs, work_bufs)
    if key in _cached:
        return _cached[key]
    W = T * NCOEF  # 4410
    groups = [list(range(i, min(i + CHUNK, NT))) for i in range(0, NT, CHUNK)]

    nc = bacc.Bacc(None)
    moc_ext = nc.declare_dram_parameter("moc", [P, RP * NCOEF], bf16, isOutput=False)
    a_ext = nc.declare_dram_parameter("a", [P, RP * NCOEF], bf16, isOutput=False)
    dyb_ext = nc.declare_dram_parameter("dyb", [P, RP * 3], bf16, isOutput=False)
    vb_ext = nc.declare_dram_parameter("vb", [P, W], bf16, isOutput=False)
    out_ext = nc.declare_dram_parameter("out", [P, 2 * NT], f32, isOutput=True)

    with tile.TileContext(nc) as tc:
        with tc.tile_pool(name="consts", bufs=1) as consts, \
             tc.tile_pool(name="io", bufs=io_bufs) as io, \
             tc.tile_pool(name="work", bufs=work_bufs) as work:
            vbt = consts.tile([P, W], bf16, name="vbt", tag="vbt")
            nc.sync.dma_start(out=vbt[:], in_=vb_ext[:])
            accs = consts.tile([P, 2 * NT], f32, name="accs", tag="accs")
            nc.vector.memset(accs[:], 0.0)

            def body():
                for g, tidx in enumerate(groups):
                    lo, hi = T * tidx[0], T * (tidx[-1] + 1)
                    rows = hi - lo
                    mo_ch = io.tile([P, rows * NCOEF], bf16, tag="mo",
                                    name=f"mo{g}")
                    nc.sync.dma_start(out=mo_ch[:],
                                      in_=moc_ext[:, lo * NCOEF:hi * NCOEF])
                    a_ch = io.tile([P, rows * NCOEF], bf16, tag="a",
                                   name=f"a{g}")
                    nc.sync.dma_start(out=a_ch[:],
                                      in_=a_ext[:, lo * NCOEF:hi * NCOEF])
                    dyb_ch = io.tile([P, rows * 3], bf16, tag="dyb",
                                     name=f"dyb{g}")
                    nc.sync.dma_start(out=dyb_ch[:],
                                      in_=dyb_ext[:, lo * 3:hi * 3])

                    for i in tidx:
                        c0 = (T * i - lo) * NCOEF
                        bc = mo_ch[:, c0:c0 + W]
                        av = a_ch[:, c0:c0 + W]
                        dyb3 = dyb_ch[:, (T * i - lo) * 3:(T * (i + 1) - lo) * 3] \
                            .rearrange("p (t c) -> p t c", c=3)

                        nc.vector.tensor_tensor(bc, bc, vbt[:], add)

                        prod = work.tile([P, W], bf16, tag="prod", name=f"p{g}_{i}")
                        nc.vector.tensor_tensor(prod[:], av, bc, mult)

                        p3 = prod[:].rearrange("p (t c) -> p t c", c=NCOEF)
                        r1 = work.tile([P, T * 8], bf16, tag="r1", name=f"r1{g}_{i}")
                        r13 = r1[:].rearrange("p (t c) -> p t c", c=8)
                        nc.vector.tensor_tensor(r1[:], p3[:, :, 0:8],
                                                p3[:, :, 8:16], add)
                        nc.vector.tensor_tensor(r13[:, :, 0:2], r13[:, :, 0:2],
                                                p3[:, :, 16:18], add)
                        r2 = work.tile([P, T * 4], bf16, tag="r2", name=f"r2{g}_{i}")
                        r23 = r2[:].rearrange("p (t c) -> p t c", c=4)
                        nc.vector.tensor_tensor(r2[:], r13[:, :, 0:4],
                                                r13[:, :, 4:8], add)
                        r3 = work.tile([P, T * 2], bf16, tag="r3", name=f"r3{g}_{i}")
                        r33 = r3[:].rearrange("p (t c) -> p t c", c=2)
                        nc.vector.tensor_tensor(r3[:], r23[:, :, 0:2],
                                                r23[:, :, 2:4], add)
                        tmp = work.tile([P, T], bf16, tag="tmp", name=f"t{g}_{i}")
                        nc.vector.tensor_tensor(tmp[:], r33[:, :, 0],
                                                r33[:, :, 1], add)

                        d_t = work.tile([P, T], bf16, tag="d", name=f"d{g}_{i}")
                        nc.scalar.copy(out=d_t[:], in_=dyb3[:, :, 0])
                        y_t = work.tile([P, T], bf16, tag="y", name=f"y{g}_{i}")
                        nc.scalar.copy(out=y_t[:], in_=dyb3[:, :, 1])
                        b_t = work.tile([P, T], bf16, tag="b", name=f"b{g}_{i}")
                        nc.scalar.copy(out=b_t[:], in_=dyb3[:, :, 2])

                        nc.vector.tensor_tensor(tmp[:], tmp[:], d_t[:], add)
                        e1 = work.tile([P, T], bf16, tag="e1", name=f"e1{g}_{i}")
                        nc.vector.tensor_tensor(e1[:], tmp[:], y_t[:], sub)
                        e2 = work.tile([P, T], bf16, tag="e2", name=f"e2{g}_{i}")
                        nc.vector.tensor_tensor(e2[:], tmp[:], b_t[:], sub)
                        sq1 = work.tile([P, T], bf16, tag="sq1", name=f"s1{g}_{i}")
                        nc.scalar.activation(sq1[:], e1[:], SQ,
                                             accum_out=accs[:, 2 * i:2 * i + 1])
                        sq2 = work.tile([P, T], bf16, tag="sq2", name=f"s2{g}_{i}")
                        nc.scalar.activation(sq2[:], e2[:], SQ,
                                             accum_out=accs[:, 2 * i + 1:2 * i + 2])

            if hw_rep:
                with tc.For_i(0, hw_rep) as _:
                    body()
            else:
                body()

            nc.sync.dma_start(out=out_ext[:], in_=accs[:])
    nc.finalize()
    _cached[key] = nc
    return nc


def _prepare_in_maps(model_output, y, A, B_tl, beta_TL):
    model_output = np.asarray(model_output, dtype=np.float32)
    y = np.asarray(y, dtype=np.float32)
    A = np.asarray(A, dtype=np.float32)
    B_tl = np.asarray(B_tl, dtype=np.float32)
    beta_TL = np.asarray(beta_TL, dtype=np.float32)

    moc = np.ascontiguousarray(model_output[:, :NCOEF])
    dyb = np.ascontiguousarray(
        np.concatenate([model_output[:, NCOEF:], y, B_tl], axis=1))  # [N, 3]
    vb = np.tile(beta_TL.astype(ml_dtypes.bfloat16), (P, T))  # [128, 4410]

    def shard(arr, ncols):
        out = []
        for i in range(N_CORES):
            lo, hi = i * R, i * R + R
            if hi <= N_TOTAL:
                sl = arr[lo:hi]
            else:
                sl = np.zeros((R, ncols), dtype=arr.dtype)
                sl[: N_TOTAL - lo] = arr[lo:]
            out.append(np.ascontiguousarray(sl).astype(ml_dtypes.bfloat16)
                       .reshape(P, RP * ncols))
        return out

    moc_s = shard(moc, NCOEF)
    a_s = shard(A, NCOEF)
    dyb_s = shard(dyb, 3)
    return [
        {"moc": moc_s[i], "a": a_s[i], "dyb": dyb_s[i], "vb": vb}
        for i in range(N_CORES)
    ]


def kernel(model_output, y, A, B_tl, beta_TL):
    nc = _build()
    in_maps = _prepare_in_maps(model_output, y, A, B_tl, beta_TL)
    res = run_bass_kernel_spmd(nc, in_maps, list(range(N_CORES)))
    total = 0.0
    for r in res.results:
        total += float(r["out"].astype(np.float64).sum())
    return np.asarray(total / N_TOTAL, dtype=np.float32)



# revision 4
# speedup vs baseline: 1.1268x; 1.1268x over previous
"""Trainium2 Bass kernel for the Tolles-Lawson custom loss.

reference:
    c = model_output[:, :18]; d = model_output[:, 18:19]
    tmp = sum(A * (beta_TL + c), axis=1, keepdims=True) + d
    L = mean((tmp - y)^2) + mean((tmp - B_tl)^2)

Sharding: pure data parallel over rows on 8 cores (R = 524,288 rows per
core, tail zero-padded; zero rows contribute 0 to both sums). Per-core
partial sums land in accs [128, 16]; the all-reduce is host-side.

Layout (per core), "transposed": rows split into 64 groups of NCG=8192;
SBUF partition p = 2*g + s holds slot s (coefficients 9s..9s+8) of
group g, rows along the free axis as 9 slabs of 512 columns per chunk.
The 18-way row-reduction runs on the TensorEngine: for each 512-column
chunk, 9 accumulating matmuls (one per coefficient slab t) with a
block-ones stationary W1 [128, 64] (W1[2g+s, g] = 1) compute
    psum[64b+g, n] = sum_s sum_t prod[2g+s, (b,t,n)]
(b = chunk parity selecting the PSUM base partition 0/64 - matmul
output base must be 0/32/64). The beta term sum_j A_ij * beta_j is
folded into 9 more matmuls with W_bt[2g+s, g] = beta[9s+t] applied to
the raw A stream, so no engine ever materializes (c + beta).
Two chunks fill a fat [128, 512] PSUM tile; the epilogue runs at full
partition width.

Engine split per fill [128, 9216] (= 65,536 rows):
  ACT : fp8->bf16 upcast of c (Copy), squares+accum of e1/e2
  DVE : prod = a * c_bf16 (2x mode), e0 = psum + d, e1/e2 = e0 - y/b
  PE  : 2 blocks x (9 beta-matmuls on a + 9 ones-matmuls on prod)
  DMA : a [128,9216] bf16, c [128,9216] fp8(e4m3), dyb [128,1536] bf16

The c stream is stored fp8 to cut HBM traffic (~31.5 MB/core/pass vs
39 MB baseline); A stays bf16 (the DVE multiply needs a 2-byte dtype
for 2x mode). Accumulation is f32 (PSUM + ACT accumulator).
"""

import numpy as np
import ml_dtypes

import concourse.bacc as bacc
import concourse.mybir as mybir
from concourse import tile
from concourse.bass_utils import run_bass_kernel_spmd

N_TOTAL = 4_000_000
NCOEF = 18
NG = 64                # row groups per core
NSLOT = 2              # coefficient slots per group
NSLAB = 9              # coefficients per slot
P = NG * NSLOT         # 128 SBUF partitions
F = 512                # chunk columns (one PSUM bank of f32)
NBLK = 2               # chunks per fill -> NBLK*NG = 128 psum partitions
FILLW = F * NBLK * NSLAB   # 9216 free elements per fill per partition
NFILL = 8              # fills per core per pass
NCHUNK = NBLK * NFILL  # 16 chunks per group
NCG = F * NCHUNK       # 8192 columns per group
R = NG * NCG           # 524288 rows per core
N_CORES = 8

C_FP8 = True           # store the c stream as float8 e4m3

f32 = mybir.dt.float32
bf16 = mybir.dt.bfloat16
fp8 = mybir.dt.float8e4
add = mybir.AluOpType.add
sub = mybir.AluOpType.subtract
mult = mybir.AluOpType.mult
COPY = mybir.ActivationFunctionType.Copy
SQ = mybir.ActivationFunctionType.Square

_cached = {}


def _build(hw_rep=0, dma_only=False, c_fp8=C_FP8):
    """hw_rep > 0 wraps the pass in a For_i hardware loop (bench only)."""
    key = (hw_rep, dma_only, c_fp8)
    if key in _cached:
        return _cached[key]
    c_dt = fp8 if c_fp8 else bf16

    nc = bacc.Bacc(None)
    a_ext = nc.declare_dram_parameter("a", [P, NFILL * FILLW], bf16,
                                      isOutput=False)
    c_ext = nc.declare_dram_parameter("c", [P, NFILL * FILLW], c_dt,
                                      isOutput=False)
    dyb_ext = nc.declare_dram_parameter("dyb", [P, NFILL * 3 * F], bf16,
                                        isOutput=False)
    # wb[:, 64*t : 64*(t+1)] = W_beta_t for t < 9; wb[:, 576:640] = W1
    wb_ext = nc.declare_dram_parameter("wb", [P, (NSLAB + 1) * NG], bf16,
                                       isOutput=False)
    out_ext = nc.declare_dram_parameter("out", [P, 2 * NFILL], f32,
                                        isOutput=True)

    with tile.TileContext(nc) as tc:
        with tc.tile_pool(name="consts", bufs=1) as consts, \
             tc.tile_pool(name="aio", bufs=3) as aio, \
             tc.tile_pool(name="cio", bufs=3) as cio, \
             tc.tile_pool(name="dio", bufs=3) as dio, \
             tc.tile_pool(name="work", bufs=2) as work, \
             tc.tile_pool(name="epi", bufs=2) as epi, \
             tc.psum_pool(name="ps", bufs=4) as pspool:
            wb_t = consts.tile([P, (NSLAB + 1) * NG], bf16, name="wb",
                               tag="wb")
            nc.sync.dma_start(out=wb_t[:], in_=wb_ext[:])
            accs = consts.tile([P, 2 * NFILL], f32, name="accs", tag="accs")
            nc.vector.memset(accs[:], 0.0)

            def wview(t):
                return wb_t[:, NG * t:NG * (t + 1)]

            def epilogue(f, ps, dyb_ch):
                d_v = dyb_ch[:, 0:F]
                y_v = dyb_ch[:, F:2 * F]
                b_v = dyb_ch[:, 2 * F:3 * F]
                e0 = epi.tile([P, F], bf16, tag="e0", name=f"e0_{f}")
                nc.vector.tensor_tensor(e0[:], ps[:], d_v, add)
                e1 = epi.tile([P, F], bf16, tag="e1", name=f"e1_{f}")
                nc.vector.tensor_tensor(e1[:], e0[:], y_v, sub)
                e2 = epi.tile([P, F], bf16, tag="e2", name=f"e2_{f}")
                nc.vector.tensor_tensor(e2[:], e0[:], b_v, sub)
                s1 = epi.tile([P, F], bf16, tag="s1", name=f"s1_{f}")
                nc.scalar.activation(s1[:], e1[:], SQ,
                                     accum_out=accs[:, 2 * f:2 * f + 1])
                s2 = epi.tile([P, F], bf16, tag="s2", name=f"s2_{f}")
                nc.scalar.activation(s2[:], e2[:], SQ,
                                     accum_out=accs[:, 2 * f + 1:2 * f + 2])

            def body():
                pend = []  # software pipeline: epilogue(f) after work(f+1)
                for f in range(NFILL):
                    a_ch = aio.tile([P, FILLW], bf16, tag="a", name=f"a{f}")
                    nc.sync.dma_start(out=a_ch[:],
                                      in_=a_ext[:, f * FILLW:(f + 1) * FILLW])
                    c_ch = cio.tile([P, FILLW], c_dt, tag="c", name=f"c{f}")
                    nc.sync.dma_start(out=c_ch[:],
                                      in_=c_ext[:, f * FILLW:(f + 1) * FILLW])
                    dyb_ch = dio.tile([P, 3 * F], bf16, tag="dyb",
                                      name=f"dyb{f}")
                    nc.sync.dma_start(
                        out=dyb_ch[:],
                        in_=dyb_ext[:, f * 3 * F:(f + 1) * 3 * F])
                    if dma_only:
                        continue

                    if c_fp8:
                        bc = work.tile([P, FILLW], bf16, tag="bc",
                                       name=f"bc{f}")
                        nc.scalar.activation(bc[:], c_ch[:], COPY)
                        c_use = bc
                    else:
                        c_use = c_ch
                    prod = work.tile([P, FILLW], bf16, tag="prod",
                                     name=f"p{f}")
                    nc.vector.tensor_tensor(prod[:], a_ch[:], c_use[:], mult)
                    ps = pspool.tile([P, F], f32, tag="ps", name=f"ps{f}")
                    for b in range(NBLK):
                        out_v = ps[NG * b:NG * (b + 1), :]
                        base = (b * NSLAB) * F
                        # beta matmuls first: they only need a_ch, so PE
                        # starts while DVE is still computing prod
                        for t in range(NSLAB):
                            nc.tensor.matmul(
                                out_v, wview(t),
                                a_ch[:, base + t * F:base + (t + 1) * F],
                                start=(t == 0), stop=False)
                        for t in range(NSLAB):
                            nc.tensor.matmul(
                                out_v, wview(NSLAB),
                                prod[:, base + t * F:base + (t + 1) * F],
                                start=False, stop=(t == NSLAB - 1))
                    pend.append((f, ps, dyb_ch))
                    if len(pend) > 1:
                        epilogue(*pend.pop(0))
                for args in pend:
                    epilogue(*args)

            if hw_rep:
                with tc.For_i(0, hw_rep) as _:
                    body()
            else:
                body()

            nc.sync.dma_start(out=out_ext[:], in_=accs[:])
    nc.finalize()
    _cached[key] = nc
    return nc


def _prepare_in_maps(model_output, y, A, B_tl, beta_TL, c_fp8=C_FP8):
    model_output = np.asarray(model_output, dtype=np.float32)
    y = np.asarray(y, dtype=np.float32)
    A = np.asarray(A, dtype=np.float32)
    B_tl = np.asarray(B_tl, dtype=np.float32)
    beta_TL = np.asarray(beta_TL, dtype=np.float32)
    c_np = ml_dtypes.float8_e4m3 if c_fp8 else ml_dtypes.bfloat16

    # wb: 9 beta-weight matrices then the ones matrix, each [128, 64]
    wb = np.zeros((P, (NSLAB + 1) * NG), dtype=ml_dtypes.bfloat16)
    for t in range(NSLAB):
        for s in range(NSLOT):
            bval = np.float32(beta_TL[NSLAB * s + t])
            for g in range(NG):
                wb[NSLOT * g + s, NG * t + g] = bval
    for g in range(NG):
        for s in range(NSLOT):
            wb[NSLOT * g + s, NG * NSLAB + g] = 1.0

    def tcoef(arr, dtype):
        # [R, 18] -> partition 2g+s, free (chunk, slab t, col n), coef 9s+t
        return np.ascontiguousarray(
            arr.reshape(NG, NCHUNK, F, NSLOT, NSLAB)
            .transpose(0, 3, 1, 4, 2).reshape(P, NFILL * FILLW)
        ).astype(dtype)

    def tfat(arr):
        # [R] -> [NFILL, 128(=64b+g), F]
        return arr.reshape(NG, NFILL, NBLK, F).transpose(1, 2, 0, 3) \
                  .reshape(NFILL, P, F)

    in_maps = []
    for i in range(N_CORES):
        lo, hi = i * R, min((i + 1) * R, N_TOTAL)
        mo_sh = np.zeros((R, NCOEF + 1), dtype=np.float32)
        mo_sh[:hi - lo] = model_output[lo:hi]
        a_sh = np.zeros((R, NCOEF), dtype=np.float32)
        a_sh[:hi - lo] = A[lo:hi]
        d_sh = mo_sh[:, NCOEF].copy()
        y_sh = np.zeros((R,), dtype=np.float32)
        y_sh[:hi - lo] = y[lo:hi, 0]
        b_sh = np.zeros((R,), dtype=np.float32)
        b_sh[:hi - lo] = B_tl[lo:hi, 0]

        a_t = tcoef(a_sh, ml_dtypes.bfloat16)
        c_t = tcoef(mo_sh[:, :NCOEF], c_np)
        dyb = np.ascontiguousarray(
            np.concatenate([tfat(d_sh), tfat(y_sh), tfat(b_sh)], axis=2)
            .transpose(1, 0, 2)
            .reshape(P, NFILL * 3 * F)).astype(ml_dtypes.bfloat16)
        in_maps.append({"a": a_t, "c": c_t, "dyb": dyb, "wb": wb})
    return in_maps


def kernel(model_output, y, A, B_tl, beta_TL):
    nc = _build()
    in_maps = _prepare_in_maps(model_output, y, A, B_tl, beta_TL)
    res = run_bass_kernel_spmd(nc, in_maps, list(range(N_CORES)))
    total = 0.0
    for r in res.results:
        total += float(r["out"].astype(np.float64).sum())
    return np.asarray(total / N_TOTAL, dtype=np.float32)


# revision 13
# speedup vs baseline: 1.4303x; 1.2694x over previous
"""Trainium2 Bass kernel for the Tolles-Lawson custom loss.

reference:
    c = model_output[:, :18]; d = model_output[:, 18:19]
    tmp = sum(A * (beta_TL + c), axis=1, keepdims=True) + d
    L = mean((tmp - y)^2) + mean((tmp - B_tl)^2)

Sharding: pure data parallel over rows on 8 cores (R = 524,288 rows per
core, tail zero-padded; zero rows contribute 0 to both sums). Per-core
partial sums land in accs [128, 18]; the all-reduce is host-side.

Layout (per core), "transposed": rows split into 64 groups of NCG=8192;
SBUF partition p = 2*g + s holds slot s (coefficients 9s..9s+8) of
group g, rows along the free axis as 9 slabs of 512 columns per chunk.
The 18-way row-reduction runs on the TensorEngine: for each 512-column
chunk, 9 accumulating matmuls (one per coefficient slab t) with a
block-ones stationary W1 [128, 64] (W1[2g+s, g] = 1) compute
    psum[64b+g, n] = sum_s sum_t prod[2g+s, (b,t,n)]
(b = chunk parity selecting the PSUM base partition 0/64 - matmul
output base must be 0/32/64). The beta term sum_j A_ij * beta_j is
folded into 9 more matmuls with W_bt[2g+s, g] = beta[9s+t] applied to
the raw A stream, so no engine ever materializes (c + beta).
Two chunks fill a fat [128, 512] PSUM tile; the epilogue runs at full
partition width.

Engine split per fill [128, 9216] (= 65,536 rows):
  ACT : fp8->bf16 upcast of c (Copy), squares+accum of e1/e2
  DVE : fp8->bf16 upcast of a (first XF8 fills), prod = a * c (2x mode),
        e0 = psum + d, e1/e2 = e0 - y/b
  PE  : 2 blocks x (9 beta-matmuls on a + 9 ones-matmuls on prod)
  DMA : a [128,9216] (fp8 first XF8 fills, bf16 after), c fp8(e4m3),
        dyb [128,1536] bf16

Precision: c is always fp8 e4m3, a is fp8 on the first XF8 fills
(~27 MB/core/pass vs 39 MB bf16 baseline; measured DMA is ~326 GB/s so
the floor is ~82 us). The per-element quantization noise is random and
washes out in the 4M-row mean (measured rel err ~1e-3 vs 2e-2 budget).

Drain taper: the benchmark (and any single launch) pays the pipeline
drain after the last DMA. The last fill is bf16 (no upcast in its
chain), its c/dyb are prefetched early, and it is processed as two
half-fills of one 512-column chunk each so the post-DMA tail is a short
[128,4608] multiply + 9 matmuls + [64,512] epilogue.
"""

import numpy as np
import ml_dtypes

import concourse.bacc as bacc
import concourse.mybir as mybir
from concourse import tile
from concourse.bass_utils import run_bass_kernel_spmd

N_TOTAL = 4_000_000
NCOEF = 18
NG = 64                # row groups per core
NSLOT = 2              # coefficient slots per group
NSLAB = 9              # coefficients per slot
P = NG * NSLOT         # 128 SBUF partitions
F = 512                # chunk columns (one PSUM bank of f32)
NBLK = 2               # chunks per fill -> NBLK*NG = 128 psum partitions
HFILL = F * NSLAB      # 4608 free elements per half-fill (one chunk)
FILLW = HFILL * NBLK   # 9216 free elements per fill per partition
NFILL = 8              # fills per core per pass
NCHUNK = NBLK * NFILL  # 16 chunks per group
NCG = F * NCHUNK       # 8192 columns per group
R = NG * NCG           # 524288 rows per core
N_CORES = 8

XF8 = 4                # leading fills whose a-stream is fp8
TAPER = True           # process the last fill as two half-fills

f32 = mybir.dt.float32
bf16 = mybir.dt.bfloat16
fp8 = mybir.dt.float8e4
add = mybir.AluOpType.add
sub = mybir.AluOpType.subtract
mult = mybir.AluOpType.mult
COPY = mybir.ActivationFunctionType.Copy
SQ = mybir.ActivationFunctionType.Square

_cached = {}


def _build(hw_rep=0, dma_only=False, probe=None, xf8=XF8, taper=TAPER):
    """hw_rep > 0 wraps the pass in a For_i hardware loop (bench only).

    probe: None = full kernel; "dma" = DMAs only.
    """
    if dma_only:
        probe = "dma"
    key = (hw_rep, probe, xf8, taper)
    if key in _cached:
        return _cached[key]

    nc = bacc.Bacc(None)
    a8_ext = c8_ext = a16_ext = None
    if xf8 > 0:
        a8_ext = nc.declare_dram_parameter("a8", [P, xf8 * FILLW], fp8,
                                           isOutput=False)
    if xf8 < NFILL:
        a16_ext = nc.declare_dram_parameter(
            "a16", [P, (NFILL - xf8) * FILLW], bf16, isOutput=False)
    c8_ext = nc.declare_dram_parameter("c8", [P, NFILL * FILLW], fp8,
                                       isOutput=False)
    dyb_ext = nc.declare_dram_parameter("dyb", [P, NFILL * 3 * F], bf16,
                                        isOutput=False)
    # wb[:, 64*t : 64*(t+1)] = W_beta_t for t < 9; wb[:, 576:640] = W1
    wb_ext = nc.declare_dram_parameter("wb", [P, (NSLAB + 1) * NG], bf16,
                                       isOutput=False)
    out_ext = nc.declare_dram_parameter("out", [P, 2 * (NFILL + 1)], f32,
                                        isOutput=True)

    with tile.TileContext(nc) as tc:
        with tc.tile_pool(name="consts", bufs=1) as consts, \
             tc.tile_pool(name="a8io", bufs=2) as a8io, \
             tc.tile_pool(name="aio", bufs=2) as aio, \
             tc.tile_pool(name="ahio", bufs=2) as ahio, \
             tc.tile_pool(name="cio", bufs=3) as cio, \
             tc.tile_pool(name="dio", bufs=4) as dio, \
             tc.tile_pool(name="work", bufs=3) as work, \
             tc.tile_pool(name="aup", bufs=2) as aupp, \
             tc.tile_pool(name="epi", bufs=2) as epi, \
             tc.psum_pool(name="ps", bufs=4) as pspool:
            wb_t = consts.tile([P, (NSLAB + 1) * NG], bf16, name="wb",
                               tag="wb")
            nc.sync.dma_start(out=wb_t[:], in_=wb_ext[:])
            accs = consts.tile([P, 2 * (NFILL + 1)], f32, name="accs",
                               tag="accs")
            nc.vector.memset(accs[:], 0.0)

            def wview(t):
                return wb_t[:, NG * t:NG * (t + 1)]

            def dma_a(f):
                if f < xf8:
                    a_ch = a8io.tile([P, FILLW], fp8, tag="a8", name=f"a{f}")
                    src = a8_ext[:, f * FILLW:(f + 1) * FILLW]
                else:
                    a_ch = aio.tile([P, FILLW], bf16, tag="a16", name=f"a{f}")
                    fo = f - xf8
                    src = a16_ext[:, fo * FILLW:(fo + 1) * FILLW]
                nc.sync.dma_start(out=a_ch[:], in_=src)
                return a_ch

            def dma_a_half(f, h):
                fo = f - xf8
                a_ch = ahio.tile([P, HFILL], bf16, tag="a16h",
                                 name=f"a{f}h{h}")
                lo = fo * FILLW + h * HFILL
                nc.sync.dma_start(out=a_ch[:], in_=a16_ext[:, lo:lo + HFILL])
                return a_ch

            def dma_c(f):
                c_ch = cio.tile([P, FILLW], fp8, tag="c", name=f"c{f}")
                nc.sync.dma_start(out=c_ch[:],
                                  in_=c8_ext[:, f * FILLW:(f + 1) * FILLW])
                return c_ch

            def dma_dyb(f):
                dyb_ch = dio.tile([P, 3 * F], bf16, tag="dyb", name=f"dyb{f}")
                nc.sync.dma_start(
                    out=dyb_ch[:], in_=dyb_ext[:, f * 3 * F:(f + 1) * 3 * F])
                return dyb_ch

            def epilogue(slot, ps, dyb_ch, p0, np_):
                """Square-accumulate (psum+d-y) and (psum+d-b) for psum
                partitions [p0, p0+np_) into accs column pair `slot`."""
                sl = slice(p0, p0 + np_)
                pv = ps[sl, :]
                d_v = dyb_ch[sl, 0:F]
                y_v = dyb_ch[sl, F:2 * F]
                b_v = dyb_ch[sl, 2 * F:3 * F]
                e0 = epi.tile([P, F], bf16, tag="e0", name=f"e0_{slot}")
                nc.vector.tensor_tensor(e0[sl, :], pv, d_v, add)
                e1 = epi.tile([P, F], bf16, tag="e1", name=f"e1_{slot}")
                nc.vector.tensor_tensor(e1[sl, :], e0[sl, :], y_v, sub)
                e2 = epi.tile([P, F], bf16, tag="e2", name=f"e2_{slot}")
                nc.vector.tensor_tensor(e2[sl, :], e0[sl, :], b_v, sub)
                s1 = epi.tile([P, F], bf16, tag="s1", name=f"s1_{slot}")
                nc.scalar.activation(
                    s1[sl, :], e1[sl, :], SQ,
                    accum_out=accs[sl, 2 * slot:2 * slot + 1])
                s2 = epi.tile([P, F], bf16, tag="s2", name=f"s2_{slot}")
                nc.scalar.activation(
                    s2[sl, :], e2[sl, :], SQ,
                    accum_out=accs[sl, 2 * slot + 1:2 * slot + 2])

            def matmuls(out_v, a_view, prod_view):
                # beta matmuls first: they only need the a stream, so PE
                # starts while DVE is still computing prod
                for t in range(NSLAB):
                    nc.tensor.matmul(out_v, wview(t),
                                     a_view[:, t * F:(t + 1) * F],
                                     start=(t == 0), stop=False)
                for t in range(NSLAB):
                    nc.tensor.matmul(out_v, wview(NSLAB),
                                     prod_view[:, t * F:(t + 1) * F],
                                     start=False, stop=(t == NSLAB - 1))

            def half_work(f, b, a_v, a_is_fp8, c_v, ps_v):
                """One chunk: upcasts, multiply, 18 matmuls into ps_v."""
                if a_is_fp8:
                    aup = aupp.tile([P, HFILL], bf16, tag="aup",
                                    name=f"au{f}_{b}")
                    nc.vector.tensor_copy(aup[:], a_v)
                    a_mm, a_mul = a_v, aup[:]  # PE reads fp8 directly
                else:
                    a_mm = a_mul = a_v
                bc = work.tile([P, HFILL], bf16, tag="bc", name=f"bc{f}_{b}")
                nc.scalar.activation(bc[:], c_v, COPY)
                prod = work.tile([P, HFILL], bf16, tag="prod",
                                 name=f"p{f}_{b}")
                nc.vector.tensor_tensor(prod[:], a_mul, bc[:], mult)
                matmuls(ps_v, a_mm, prod[:])

            def fill_work(f, a_ch, c_ch):
                ps = pspool.tile([P, F], f32, tag="ps", name=f"ps{f}")
                for b in range(NBLK):
                    base = b * HFILL
                    half_work(f, b, a_ch[:, base:base + HFILL], f < xf8,
                              c_ch[:, base:base + HFILL],
                              ps[NG * b:NG * (b + 1), :])
                return ps

            def body():
                n_main = NFILL - 1 if taper else NFILL
                pend = []  # software pipeline: epilogue(f) after work(f+1)
                c_last = dyb_last = None
                for f in range(n_main):
                    a_ch = dma_a(f)
                    c_ch = dma_c(f)
                    dyb_ch = dma_dyb(f)
                    if f == n_main - 2 and taper:
                        # prefetch the taper fill's small streams so its
                        # a-halves are the only post-prefetch DMAs
                        c_last = dma_c(NFILL - 1)
                        dyb_last = dma_dyb(NFILL - 1)
                    if probe == "dma":
                        continue
                    ps = fill_work(f, a_ch, c_ch)
                    pend.append((f, ps, dyb_ch))
                    if len(pend) > 1:
                        fo, pso, dybo = pend.pop(0)
                        epilogue(fo, pso, dybo, 0, P)
                if taper:
                    fl = NFILL - 1
                    if probe == "dma":
                        for h in range(NBLK):
                            dma_a_half(fl, h)
                    else:
                        psl = pspool.tile([P, F], f32, tag="ps", name="psl")
                        for h in range(NBLK):
                            a_h = dma_a_half(fl, h)
                            half_work(fl, h, a_h[:], False,
                                      c_last[:, h * HFILL:(h + 1) * HFILL],
                                      psl[NG * h:NG * (h + 1), :])
                            if pend:
                                fo, pso, dybo = pend.pop(0)
                                epilogue(fo, pso, dybo, 0, P)
                            epilogue(fl + h, psl, dyb_last, NG * h, NG)
                for args in pend:
                    f, pso, dybo = args
                    epilogue(f, pso, dybo, 0, P)

            if hw_rep:
                with tc.For_i(0, hw_rep) as _:
                    body()
            else:
                body()

            nc.sync.dma_start(out=out_ext[:], in_=accs[:])
    nc.finalize()
    _cached[key] = nc
    return nc


def _prepare_in_maps(model_output, y, A, B_tl, beta_TL, xf8=XF8):
    model_output = np.asarray(model_output, dtype=np.float32)
    y = np.asarray(y, dtype=np.float32)
    A = np.asarray(A, dtype=np.float32)
    B_tl = np.asarray(B_tl, dtype=np.float32)
    beta_TL = np.asarray(beta_TL, dtype=np.float32)

    # wb: 9 beta-weight matrices then the ones matrix, each [128, 64]
    wb = np.zeros((P, (NSLAB + 1) * NG), dtype=np.float32)
    g_idx = np.arange(NG)
    for t in range(NSLAB):
        for s in range(NSLOT):
            wb[NSLOT * g_idx + s, NG * t + g_idx] = beta_TL[NSLAB * s + t]
    for s in range(NSLOT):
        wb[NSLOT * g_idx + s, NG * NSLAB + g_idx] = 1.0
    wb = wb.astype(ml_dtypes.bfloat16)

    def tcoef(arr):
        # [R, 18] -> partition 2g+s, free (chunk, slab t, col n), coef 9s+t
        return np.ascontiguousarray(
            arr.reshape(NG, NCHUNK, F, NSLOT, NSLAB)
            .transpose(0, 3, 1, 4, 2).reshape(P, NFILL * FILLW))

    def tfat(arr):
        # [R] -> [NFILL, 128(=64b+g), F]
        return arr.reshape(NG, NFILL, NBLK, F).transpose(1, 2, 0, 3) \
                  .reshape(NFILL, P, F)

    in_maps = []
    for i in range(N_CORES):
        lo, hi = i * R, min((i + 1) * R, N_TOTAL)
        mo_sh = np.zeros((R, NCOEF + 1), dtype=np.float32)
        mo_sh[:hi - lo] = model_output[lo:hi]
        a_sh = np.zeros((R, NCOEF), dtype=np.float32)
        a_sh[:hi - lo] = A[lo:hi]
        d_sh = mo_sh[:, NCOEF].copy()
        y_sh = np.zeros((R,), dtype=np.float32)
        y_sh[:hi - lo] = y[lo:hi, 0]
        b_sh = np.zeros((R,), dtype=np.float32)
        b_sh[:hi - lo] = B_tl[lo:hi, 0]

        a_t = tcoef(a_sh)
        c_t = tcoef(mo_sh[:, :NCOEF]).astype(ml_dtypes.float8_e4m3)
        dyb = np.ascontiguousarray(
            np.concatenate([tfat(d_sh), tfat(y_sh), tfat(b_sh)], axis=2)
            .transpose(1, 0, 2)
            .reshape(P, NFILL * 3 * F)).astype(ml_dtypes.bfloat16)
        m = {"c8": c_t, "dyb": dyb, "wb": wb}
        if xf8 > 0:
            m["a8"] = np.ascontiguousarray(
                a_t[:, :xf8 * FILLW]).astype(ml_dtypes.float8_e4m3)
        if xf8 < NFILL:
            m["a16"] = np.ascontiguousarray(
                a_t[:, xf8 * FILLW:]).astype(ml_dtypes.bfloat16)
        in_maps.append(m)
    return in_maps


def kernel(model_output, y, A, B_tl, beta_TL):
    nc = _build()
    in_maps = _prepare_in_maps(model_output, y, A, B_tl, beta_TL)
    res = run_bass_kernel_spmd(nc, in_maps, list(range(N_CORES)))
    total = 0.0
    for r in res.results:
        total += float(r["out"].astype(np.float64).sum())
    return np.asarray(total / N_TOTAL, dtype=np.float32)


# revision 18
# speedup vs baseline: 1.4756x; 1.0317x over previous
"""Trainium2 Bass kernel for the Tolles-Lawson custom loss.

reference:
    c = model_output[:, :18]; d = model_output[:, 18:19]
    tmp = sum(A * (beta_TL + c), axis=1, keepdims=True) + d
    L = mean((tmp - y)^2) + mean((tmp - B_tl)^2)

Sharding: pure data parallel over rows on 8 cores (R = 524,288 rows per
core, tail zero-padded; zero rows contribute 0 to both sums). Per-core
partial sums land in accs [128, 18]; the all-reduce is host-side.

Layout (per core), "transposed": rows split into 64 groups of NCG=8192;
SBUF partition p = 2*g + s holds slot s (coefficients 9s..9s+8) of
group g, rows along the free axis as 9 slabs of 512 columns per chunk.
The 18-way row-reduction runs on the TensorEngine: for each 512-column
chunk, 9 accumulating matmuls (one per coefficient slab t) with a
block-ones stationary W1 [128, 64] (W1[2g+s, g] = 1) compute
    psum[64b+g, n] = sum_s sum_t prod[2g+s, (b,t,n)]
(b = chunk parity selecting the PSUM base partition 0/64 - matmul
output base must be 0/32/64). The beta term sum_j A_ij * beta_j is
folded into 9 more matmuls with W_bt[2g+s, g] = beta[9s+t] applied to
the raw A stream, so no engine ever materializes (c + beta).
Two chunks fill a fat [128, 512] PSUM tile; the epilogue runs at full
partition width.

Engine split per fill [128, 9216] (= 65,536 rows):
  ACT : fp8->bf16 upcast of c (Copy), squares+accum of e1/e2
  DVE : fp8->bf16 upcast of a (first XF8 fills), prod = a * c (2x mode),
        e0 = psum + d, e1/e2 = e0 - y/b
  PE  : 2 blocks x (9 beta-matmuls on a + 9 ones-matmuls on prod)
  DMA : a [128,9216] (fp8 first XF8 fills, bf16 after), c fp8(e4m3),
        dyb [128,1536] bf16

Precision: c is always fp8 e4m3, a is fp8 on the first XF8 fills
(~27 MB/core/pass vs 39 MB bf16 baseline; measured DMA is ~326 GB/s so
the floor is ~82 us). The per-element quantization noise is random and
washes out in the 4M-row mean (measured rel err ~1e-3 vs 2e-2 budget).

Drain taper: the benchmark (and any single launch) pays the pipeline
drain after the last DMA. The last fill is bf16 (no upcast in its
chain), its c/dyb are prefetched early, and it is processed as two
half-fills of one 512-column chunk each so the post-DMA tail is a short
[128,4608] multiply + 9 matmuls + [64,512] epilogue.
"""

import numpy as np
import ml_dtypes

import concourse.bacc as bacc
import concourse.mybir as mybir
from concourse import tile
from concourse.bass_utils import run_bass_kernel_spmd

N_TOTAL = 4_000_000
NCOEF = 18
NG = 64                # row groups per core
NSLOT = 2              # coefficient slots per group
NSLAB = 9              # coefficients per slot
P = NG * NSLOT         # 128 SBUF partitions
F = 512                # chunk columns (one PSUM bank of f32)
NBLK = 2               # chunks per fill -> NBLK*NG = 128 psum partitions
HFILL = F * NSLAB      # 4608 free elements per half-fill (one chunk)
FILLW = HFILL * NBLK   # 9216 free elements per fill per partition
NFILL = 8              # fills per core per pass
NCHUNK = NBLK * NFILL  # 16 chunks per group
NCG = F * NCHUNK       # 8192 columns per group
R = NG * NCG           # 524288 rows per core
N_CORES = 8

XF8 = 4                # number of fills whose a-stream is fp8
TAPER = True           # process the last fill as two half-fills


def fp8_fills(xf8):
    """Spread the fp8-a fills among fills 0..NFILL-2 (the taper fill is
    always bf16) so heavy upcast fills alternate with light ones."""
    if xf8 <= 0:
        return []
    return sorted(set(
        int(round(i * (NFILL - 2) / max(xf8 - 1, 1))) for i in range(xf8)))

f32 = mybir.dt.float32
bf16 = mybir.dt.bfloat16
fp8 = mybir.dt.float8e4
add = mybir.AluOpType.add
sub = mybir.AluOpType.subtract
mult = mybir.AluOpType.mult
COPY = mybir.ActivationFunctionType.Copy
SQ = mybir.ActivationFunctionType.Square

_cached = {}


def _build(hw_rep=0, dma_only=False, probe=None, xf8=XF8, taper=TAPER):
    """hw_rep > 0 wraps the pass in a For_i hardware loop (bench only).

    probe: None = full kernel; "dma" = DMAs only.
    """
    if dma_only:
        probe = "dma"
    key = (hw_rep, probe, xf8, taper)
    if key in _cached:
        return _cached[key]
    f8set = set(fp8_fills(xf8))
    a8_slot = {f: i for i, f in enumerate(sorted(f8set))}
    a16_slot = {f: i for i, f in
                enumerate(f for f in range(NFILL) if f not in f8set)}

    nc = bacc.Bacc(None)
    a8_ext = c8_ext = a16_ext = None
    if xf8 > 0:
        a8_ext = nc.declare_dram_parameter("a8", [P, xf8 * FILLW], fp8,
                                           isOutput=False)
    if xf8 < NFILL:
        a16_ext = nc.declare_dram_parameter(
            "a16", [P, (NFILL - xf8) * FILLW], bf16, isOutput=False)
    c8_ext = nc.declare_dram_parameter("c8", [P, NFILL * FILLW], fp8,
                                       isOutput=False)
    dyb_ext = nc.declare_dram_parameter("dyb", [P, NFILL * 3 * F], bf16,
                                        isOutput=False)
    # wb[:, 64*t : 64*(t+1)] = W_beta_t for t < 9; wb[:, 576:640] = W1
    wb_ext = nc.declare_dram_parameter("wb", [P, (NSLAB + 1) * NG], bf16,
                                       isOutput=False)
    out_ext = nc.declare_dram_parameter("out", [P, 2 * (NFILL + 1)], f32,
                                        isOutput=True)

    with tile.TileContext(nc) as tc:
        with tc.tile_pool(name="consts", bufs=1) as consts, \
             tc.tile_pool(name="a8io", bufs=2) as a8io, \
             tc.tile_pool(name="aio", bufs=2) as aio, \
             tc.tile_pool(name="ahio", bufs=2) as ahio, \
             tc.tile_pool(name="cio", bufs=3) as cio, \
             tc.tile_pool(name="dio", bufs=4) as dio, \
             tc.tile_pool(name="work", bufs=3) as work, \
             tc.tile_pool(name="aup", bufs=2) as aupp, \
             tc.tile_pool(name="epi", bufs=2) as epi, \
             tc.psum_pool(name="ps", bufs=4) as pspool:
            wb_t = consts.tile([P, (NSLAB + 1) * NG], bf16, name="wb",
                               tag="wb")
            nc.sync.dma_start(out=wb_t[:], in_=wb_ext[:])
            accs = consts.tile([P, 2 * (NFILL + 1)], f32, name="accs",
                               tag="accs")
            nc.vector.memset(accs[:], 0.0)

            def wview(t):
                return wb_t[:, NG * t:NG * (t + 1)]

            def dma_a(f):
                if f in f8set:
                    a_ch = a8io.tile([P, FILLW], fp8, tag="a8", name=f"a{f}")
                    fo = a8_slot[f]
                    src = a8_ext[:, fo * FILLW:(fo + 1) * FILLW]
                else:
                    a_ch = aio.tile([P, FILLW], bf16, tag="a16", name=f"a{f}")
                    fo = a16_slot[f]
                    src = a16_ext[:, fo * FILLW:(fo + 1) * FILLW]
                nc.sync.dma_start(out=a_ch[:], in_=src)
                return a_ch

            def dma_a_half(f, h):
                fo = a16_slot[f]
                a_ch = ahio.tile([P, HFILL], bf16, tag="a16h",
                                 name=f"a{f}h{h}")
                lo = fo * FILLW + h * HFILL
                nc.sync.dma_start(out=a_ch[:], in_=a16_ext[:, lo:lo + HFILL])
                return a_ch

            def dma_c(f):
                c_ch = cio.tile([P, FILLW], fp8, tag="c", name=f"c{f}")
                nc.sync.dma_start(out=c_ch[:],
                                  in_=c8_ext[:, f * FILLW:(f + 1) * FILLW])
                return c_ch

            def dma_dyb(f):
                dyb_ch = dio.tile([P, 3 * F], bf16, tag="dyb", name=f"dyb{f}")
                nc.sync.dma_start(
                    out=dyb_ch[:], in_=dyb_ext[:, f * 3 * F:(f + 1) * 3 * F])
                return dyb_ch

            def epilogue(slot, ps, dyb_ch, p0, np_):
                """Square-accumulate (psum+d-y) and (psum+d-b) for psum
                partitions [p0, p0+np_) into accs column pair `slot`."""
                sl = slice(p0, p0 + np_)
                pv = ps[sl, :]
                d_v = dyb_ch[sl, 0:F]
                y_v = dyb_ch[sl, F:2 * F]
                b_v = dyb_ch[sl, 2 * F:3 * F]
                e0 = epi.tile([P, F], bf16, tag="e0", name=f"e0_{slot}")
                nc.vector.tensor_tensor(e0[sl, :], pv, d_v, add)
                e1 = epi.tile([P, F], bf16, tag="e1", name=f"e1_{slot}")
                nc.vector.tensor_tensor(e1[sl, :], e0[sl, :], y_v, sub)
                e2 = epi.tile([P, F], bf16, tag="e2", name=f"e2_{slot}")
                nc.vector.tensor_tensor(e2[sl, :], e0[sl, :], b_v, sub)
                s1 = epi.tile([P, F], bf16, tag="s1", name=f"s1_{slot}")
                nc.scalar.activation(
                    s1[sl, :], e1[sl, :], SQ,
                    accum_out=accs[sl, 2 * slot:2 * slot + 1])
                s2 = epi.tile([P, F], bf16, tag="s2", name=f"s2_{slot}")
                nc.scalar.activation(
                    s2[sl, :], e2[sl, :], SQ,
                    accum_out=accs[sl, 2 * slot + 1:2 * slot + 2])

            def matmuls(out_v, a_view, prod_view):
                # beta matmuls first: they only need the a stream, so PE
                # starts while DVE is still computing prod
                for t in range(NSLAB):
                    nc.tensor.matmul(out_v, wview(t),
                                     a_view[:, t * F:(t + 1) * F],
                                     start=(t == 0), stop=False)
                for t in range(NSLAB):
                    nc.tensor.matmul(out_v, wview(NSLAB),
                                     prod_view[:, t * F:(t + 1) * F],
                                     start=False, stop=(t == NSLAB - 1))

            def half_work(f, b, a_v, a_is_fp8, c_v, ps_v):
                """One chunk: upcasts, multiply, 18 matmuls into ps_v."""
                if a_is_fp8:
                    aup = aupp.tile([P, HFILL], bf16, tag="aup",
                                    name=f"au{f}_{b}")
                    nc.vector.tensor_copy(aup[:], a_v)
                    a_mm, a_mul = a_v, aup[:]  # PE reads fp8 directly
                else:
                    a_mm = a_mul = a_v
                bc = work.tile([P, HFILL], bf16, tag="bc", name=f"bc{f}_{b}")
                nc.scalar.activation(bc[:], c_v, COPY)
                prod = work.tile([P, HFILL], bf16, tag="prod",
                                 name=f"p{f}_{b}")
                nc.vector.tensor_tensor(prod[:], a_mul, bc[:], mult)
                matmuls(ps_v, a_mm, prod[:])

            def fill_work(f, a_ch, c_ch):
                ps = pspool.tile([P, F], f32, tag="ps", name=f"ps{f}")
                for b in range(NBLK):
                    base = b * HFILL
                    half_work(f, b, a_ch[:, base:base + HFILL], f in f8set,
                              c_ch[:, base:base + HFILL],
                              ps[NG * b:NG * (b + 1), :])
                return ps

            def body():
                n_main = NFILL - 1 if taper else NFILL
                pend = []  # software pipeline: epilogue(f) after work(f+1)
                c_last = dyb_last = None
                for f in range(n_main):
                    a_ch = dma_a(f)
                    c_ch = dma_c(f)
                    dyb_ch = dma_dyb(f)
                    if f == n_main - 2 and taper:
                        # prefetch the taper fill's small streams so its
                        # a-halves are the only post-prefetch DMAs
                        c_last = dma_c(NFILL - 1)
                        dyb_last = dma_dyb(NFILL - 1)
                    if probe == "dma":
                        continue
                    ps = fill_work(f, a_ch, c_ch)
                    pend.append((f, ps, dyb_ch))
                    if len(pend) > 1:
                        fo, pso, dybo = pend.pop(0)
                        epilogue(fo, pso, dybo, 0, P)
                if taper:
                    fl = NFILL - 1
                    if probe == "dma":
                        for h in range(NBLK):
                            dma_a_half(fl, h)
                    else:
                        psl = pspool.tile([P, F], f32, tag="ps", name="psl")
                        for h in range(NBLK):
                            a_h = dma_a_half(fl, h)
                            half_work(fl, h, a_h[:], False,
                                      c_last[:, h * HFILL:(h + 1) * HFILL],
                                      psl[NG * h:NG * (h + 1), :])
                            if pend:
                                fo, pso, dybo = pend.pop(0)
                                epilogue(fo, pso, dybo, 0, P)
                            epilogue(fl + h, psl, dyb_last, NG * h, NG)
                for args in pend:
                    f, pso, dybo = args
                    epilogue(f, pso, dybo, 0, P)

            if hw_rep:
                with tc.For_i(0, hw_rep) as _:
                    body()
            else:
                body()

            nc.sync.dma_start(out=out_ext[:], in_=accs[:])
    nc.finalize()
    _cached[key] = nc
    return nc


def _prepare_in_maps(model_output, y, A, B_tl, beta_TL, xf8=XF8):
    model_output = np.asarray(model_output, dtype=np.float32)
    y = np.asarray(y, dtype=np.float32)
    A = np.asarray(A, dtype=np.float32)
    B_tl = np.asarray(B_tl, dtype=np.float32)
    beta_TL = np.asarray(beta_TL, dtype=np.float32)

    # wb: 9 beta-weight matrices then the ones matrix, each [128, 64]
    wb = np.zeros((P, (NSLAB + 1) * NG), dtype=np.float32)
    g_idx = np.arange(NG)
    for t in range(NSLAB):
        for s in range(NSLOT):
            wb[NSLOT * g_idx + s, NG * t + g_idx] = beta_TL[NSLAB * s + t]
    for s in range(NSLOT):
        wb[NSLOT * g_idx + s, NG * NSLAB + g_idx] = 1.0
    wb = wb.astype(ml_dtypes.bfloat16)

    def tcoef(arr):
        # [R, 18] -> partition 2g+s, free (chunk, slab t, col n), coef 9s+t
        return np.ascontiguousarray(
            arr.reshape(NG, NCHUNK, F, NSLOT, NSLAB)
            .transpose(0, 3, 1, 4, 2).reshape(P, NFILL * FILLW))

    def tfat(arr):
        # [R] -> [NFILL, 128(=64b+g), F]
        return arr.reshape(NG, NFILL, NBLK, F).transpose(1, 2, 0, 3) \
                  .reshape(NFILL, P, F)

    in_maps = []
    for i in range(N_CORES):
        lo, hi = i * R, min((i + 1) * R, N_TOTAL)
        mo_sh = np.zeros((R, NCOEF + 1), dtype=np.float32)
        mo_sh[:hi - lo] = model_output[lo:hi]
        a_sh = np.zeros((R, NCOEF), dtype=np.float32)
        a_sh[:hi - lo] = A[lo:hi]
        d_sh = mo_sh[:, NCOEF].copy()
        y_sh = np.zeros((R,), dtype=np.float32)
        y_sh[:hi - lo] = y[lo:hi, 0]
        b_sh = np.zeros((R,), dtype=np.float32)
        b_sh[:hi - lo] = B_tl[lo:hi, 0]

        a_t = tcoef(a_sh)
        c_t = tcoef(mo_sh[:, :NCOEF]).astype(ml_dtypes.float8_e4m3)
        dyb = np.ascontiguousarray(
            np.concatenate([tfat(d_sh), tfat(y_sh), tfat(b_sh)], axis=2)
            .transpose(1, 0, 2)
            .reshape(P, NFILL * 3 * F)).astype(ml_dtypes.bfloat16)
        m = {"c8": c_t, "dyb": dyb, "wb": wb}
        f8 = fp8_fills(xf8)
        f16 = [f for f in range(NFILL) if f not in f8]
        af = a_t.reshape(P, NFILL, FILLW)
        if f8:
            m["a8"] = np.ascontiguousarray(
                af[:, f8, :].reshape(P, -1)).astype(ml_dtypes.float8_e4m3)
        if f16:
            m["a16"] = np.ascontiguousarray(
                af[:, f16, :].reshape(P, -1)).astype(ml_dtypes.bfloat16)
        in_maps.append(m)
    return in_maps


def kernel(model_output, y, A, B_tl, beta_TL):
    nc = _build()
    in_maps = _prepare_in_maps(model_output, y, A, B_tl, beta_TL)
    res = run_bass_kernel_spmd(nc, in_maps, list(range(N_CORES)))
    total = 0.0
    for r in res.results:
        total += float(r["out"].astype(np.float64).sum())
    return np.asarray(total / N_TOTAL, dtype=np.float32)
